# revision 15
# baseline (speedup 1.0000x reference)
"""Expert-choice MoE kernel for 8 Trainium2 NeuronCores (Bass/Tile).

Distribution: expert-parallel, one expert per core.
  - gate: each core computes fp32 scores z = x_shard @ Wg for its 1/8 token
    shard, AllToAll -> full (E, N) scores on every core (core c gets expert c).
  - top-k (k=2048 of N=8192) per expert: exact threshold via fp32 bisection
    (34 steps) on the global count, then index compaction with the gpsimd
    sparse_gather ucode op (SBUF-only stream compaction) + one PE transpose.
  - dispatch: indirect-DMA gather of the 2048 selected token rows (bf16).
  - expert FFN in bf16 (fp32 accumulation), erf-Gelu on the scalar engine,
    fp32 gate multiply on the way out, cast to bf16.
  - combine: scatter rows into a per-core dense bf16 (N, H) buffer,
    ReduceScatter (add) across the 8 cores -> each core owns one token shard
    of the output (bf16, upcast on host).
"""

import sys

for _p in ("/opt/trn_rl_repo",):
    if _p not in sys.path:
        sys.path.insert(0, _p)

import numpy as np
import ml_dtypes

import concourse.bass as bass
import concourse.mybir as mybir
import concourse.tile as tile
import bass_rust

# ---------------------------------------------------------------------------
# Patch: this walrus build rejects >1 sync-wait on the SP Drain that
# TileContext emits at kernel exit. Split the global-clock waits across
# several drains (1 wait each).
# ---------------------------------------------------------------------------
from concourse.vector_clock import ScopedClock

_MAX_DRAIN_WAITS = 1


def _patched_drain_and_barrier(self, tick_clock, wait_clock):
    nc = self.nc
    probe = nc.sync.drain()
    wait_clock.add_sem_waits(probe.ins, ScopedClock({None: tick_clock.global_clock}))
    si = probe.ins.sync_info
    waits = list(si.on_wait or []) if si is not None else []
    if len(waits) > _MAX_DRAIN_WAITS:
        probe.ins.sync_info = mybir.SyncInfo(
            on_wait=waits[:_MAX_DRAIN_WAITS],
            on_update=list(si.on_update or []),
        )
        for i in range(_MAX_DRAIN_WAITS, len(waits), _MAX_DRAIN_WAITS):
            extra = nc.sync.drain()
            extra.ins.sync_info = mybir.SyncInfo(
                on_wait=waits[i : i + _MAX_DRAIN_WAITS], on_update=[]
            )
    nc.all_engine_barrier()
    assert self.sems is not None
    popped = nc._tile_sem_poison_stack.pop()
    assert popped is self._sem_poison
    nc.clear_and_free_semaphores(list(self.sems.allocated().values()))
    nc.all_engine_barrier()


tile.TileContext._drain_and_barrier = _patched_drain_and_barrier

_WSPLIT_LIMIT = 1
_wsplit_ctr = [0]


def _split_excess_waits(nc, limit=_WSPLIT_LIMIT):
    """This walrus build encodes at most `limit` sync-wait commands per
    instruction; hoist excess waits onto same-engine Drain instructions
    inserted immediately before (per-engine streams execute in order)."""
    f = nc.m.functions[0]
    for b in f.blocks:
        insts = b.instructions
        out = []
        changed = False
        for inst in insts:
            si = getattr(inst, "sync_info", None)
            waits = list(si.on_wait or []) if si is not None else []
            eng = getattr(inst, "engine", None)
            if len(waits) > limit and eng is not None and \
                    eng != mybir.EngineType.Unassigned:
                keep = waits[-limit:]
                extra = waits[:-limit]
                for i in range(0, len(extra), limit):
                    d = mybir.InstDrain(
                        name=f"WSPLIT-{_wsplit_ctr[0]}", ins=[], outs=[])
                    _wsplit_ctr[0] += 1
                    d.engine = eng
                    d.sync_info = mybir.SyncInfo(
                        on_wait=extra[i:i + limit], on_update=[])
                    out.append(d)
                    nc.register_instruction(d, overwrite=True)
                inst.sync_info = mybir.SyncInfo(
                    on_wait=keep, on_update=list(si.on_update or []))
                changed = True
            out.append(inst)
        if changed:
            b.instructions = out


def _insert_library_loads(nc):
    from concourse.library_config import all_libraries, standard
    m = {}
    for lib in all_libraries:
        for t in lib.instructions:
            m[t] = m.get(t, 0) | (1 << lib.index)
    bass_rust.insert_library_loads(nc, m, len(all_libraries), standard.index)
    bass_rust.codegen_inst_isa_subclasses(nc)


dt = mybir.dt
Alu = mybir.AluOpType
Act = mybir.ActivationFunctionType

N_CORES = 8

# problem dims (full size; can be shrunk for simulation)
FULL = dict(N=8192, H=1024, FF=4096, E=8, K=2048)


def build_moe_nc(N=8192, H=1024, FF=4096, E=8, K=2048, TOKG=512, act=None):
    """Build the SPMD Bass program (same program on all 8 cores)."""
    assert E == N_CORES
    P = N // N_CORES          # tokens per shard
    HC = H // 128             # h chunks
    FC = FF // 128            # ff chunks
    NG = K // TOKG            # token groups
    SUBS = TOKG // 128        # 128-token subtiles per group
    NCOLS = K // 128          # total 128-token subtiles
    ZF = N // 128             # free size of the [128, ZF] score layout
    F16 = N // 16             # free size of the [16, F16] score layout
    KOUT = K // 16 + 32       # sparse_gather output cols (slack for ties)
    assert K % TOKG == 0 and TOKG % 128 == 0 and P % 128 == 0
    assert K == NCOLS * 128 and NCOLS <= 128
    if act is None:
        act = Act.Gelu
    NSTEP = min(512, H)
    # multi-probe bisection: NPROBE interior probes shrink the bracket by
    # (NPROBE+1)x per round; 12 rounds from +-16 -> 4.7e-10 < fp32 ulp of
    # the threshold (~3e-8), so the final count is exactly K (absent ties).
    NPROBE = 7
    BISECT_ROUNDS = 12
    BISECT_BOUND = 16.0

    nc = bass.Bass(num_devices=N_CORES)

    # ---- I/O ----
    xT_s = nc.dram_tensor("xT_s", [H, P], dt.float32, kind="ExternalInput")
    x_bf = nc.dram_tensor("x_bf", [N, H], dt.bfloat16, kind="ExternalInput")
    Wg_d = nc.dram_tensor("Wg", [H, E], dt.float32, kind="ExternalInput")
    W1_d = nc.dram_tensor("W1", [H, FF], dt.bfloat16, kind="ExternalInput")
    W2_d = nc.dram_tensor("W2", [FF, H], dt.bfloat16, kind="ExternalInput")
    b1_d = nc.dram_tensor("b1", [1, FF], dt.float32, kind="ExternalInput")
    b2_d = nc.dram_tensor("b2", [1, H], dt.float32, kind="ExternalInput")
    y_d = nc.dram_tensor("y", [P, H], dt.bfloat16, kind="ExternalOutput")

    # ---- internal DRAM ----
    z_loc_d = nc.dram_tensor("z_loc", [E, P], dt.float32)
    z_e_d = nc.dram_tensor("z_e", [N_CORES, P], dt.float32)
    dense_d = nc.dram_tensor("dense", [N, H], dt.bfloat16)
    rs_out_d = nc.dram_tensor("rs_out", [P, H], dt.bfloat16)

    groups = [list(range(N_CORES))]

    with tile.TileContext(nc) as tc:
        with (
            tc.tile_pool(name="const", bufs=1) as const_pool,
            tc.tile_pool(name="w", bufs=1) as w_pool,
            tc.tile_pool(name="psum1", bufs=2, space="PSUM") as psum1_pool,
            tc.tile_pool(name="psum2", bufs=2, space="PSUM") as psum2_pool,
            tc.tile_pool(name="ptrans", bufs=2, space="PSUM") as ptrans_pool,
        ):
            # ---------------- persistent constants ----------------
            ones1 = const_pool.tile([1, 128], dt.float32)
            nc.vector.memset(ones1[:], 1.0)

            # f - p iota: identity masks
            fmp = const_pool.tile([128, 128], dt.int32)
            nc.gpsimd.iota(fmp[:], pattern=[[1, 128]], base=0,
                           channel_multiplier=-1)
            fmp_f = const_pool.tile([128, 128], dt.float32)
            nc.vector.tensor_copy(fmp_f[:], fmp[:])
            ident_bf = const_pool.tile([128, 128], dt.bfloat16)
            nc.vector.tensor_scalar(ident_bf[:], fmp_f[:], 0.0, None,
                                    op0=Alu.is_equal)
            ident16 = const_pool.tile([16, 16], dt.float32)
            nc.vector.tensor_scalar(ident16[:], fmp_f[0:16, 0:16], 0.0, None,
                                    op0=Alu.is_equal)
            ones128 = const_pool.tile([128, 128], dt.float32)
            nc.vector.memset(ones128[:], 1.0)

            # b2 broadcast [128, H] (constant along tokens)
            b2_sb = const_pool.tile([1, H], dt.float32)
            nc.sync.dma_start(b2_sb[:], b2_d[:])
            b2_ps = psum2_pool.tile([128, H], dt.float32, tag="ps2")
            for hh in range(0, H, NSTEP):
                nc.tensor.matmul(b2_ps[:, hh:hh + NSTEP], ones1[:],
                                 b2_sb[:, hh:hh + NSTEP], start=True, stop=True)
            b2_bcast = const_pool.tile([128, H], dt.float32)
            nc.vector.tensor_copy(b2_bcast[:], b2_ps[:])

            # b1 per-partition [128, FC]
            b1_pp = const_pool.tile([128, FC], dt.float32)
            nc.sync.dma_start(
                b1_pp[:], b1_d[:].rearrange("o (c p) -> (o p) c", p=128))

            zero_row = const_pool.tile([128, H], dt.bfloat16)
            nc.vector.memset(zero_row[:], 0.0)

            # persistent routing outputs (filled by the gate phase)
            ids_pp = const_pool.tile([128, NCOLS], dt.int32)
            g_pp = const_pool.tile([128, NCOLS], dt.float32)

            # ================= gate phase (scoped pool) ================
            with (
                tc.tile_pool(name="gate", bufs=1) as gate_pool,
                tc.tile_pool(name="small", bufs=2) as small_pool,
            ):
                # gate inputs first: they head the critical path
                xT_sb = gate_pool.tile([128, HC, P], dt.float32)
                for ci in range(HC):
                    nc.sync.dma_start(
                        xT_sb[:, ci, :], xT_s[ci * 128:(ci + 1) * 128, :])
                wg_sb = gate_pool.tile([128, HC, E], dt.float32)
                nc.sync.dma_start(
                    wg_sb[:], Wg_d[:].rearrange("(c p) e -> p c e", p=128))

                # ---- expert weights stream on the Activation engine's DMA
                # queue so they never block the latency-critical gate DMAs
                # on the sync queue ----
                w1_sb = w_pool.tile([128, HC, FF], dt.bfloat16)
                for ci in range(HC):
                    nc.scalar.dma_start(
                        w1_sb[:, ci, :], W1_d[ci * 128:(ci + 1) * 128, :])
                w2_sb = w_pool.tile([128, FC, H], dt.bfloat16)
                for fc in range(FC):
                    nc.scalar.dma_start(
                        w2_sb[:, fc, :], W2_d[fc * 128:(fc + 1) * 128, :])

                z_sb_loc = gate_pool.tile([E, P], dt.float32)
                for t0 in range(0, P, 512):
                    zw = min(512, P - t0)
                    z_ps = psum1_pool.tile([E, 512], dt.float32, tag="ps1")
                    for ci in range(HC):
                        nc.tensor.matmul(z_ps[:, :zw], wg_sb[:, ci, :],
                                         xT_sb[:, ci, t0:t0 + zw],
                                         start=(ci == 0), stop=(ci == HC - 1))
                    nc.vector.tensor_copy(z_sb_loc[:, t0:t0 + zw],
                                          z_ps[:, :zw])
                nc.sync.dma_start(z_loc_d[:], z_sb_loc[:])

                # core c receives every shard's scores for expert c
                nc.gpsimd.collective_compute(
                    "AllToAll", Alu.bypass, replica_groups=groups,
                    ins=[z_loc_d[:]], outs=[z_e_d[:]],
                )

                z_sb = gate_pool.tile([128, ZF], dt.float32)
                nc.sync.dma_start(
                    z_sb[:], z_e_d[:].rearrange("q t -> (q t)").rearrange(
                        "(p f) -> p f", p=128))
                # contiguous per-partition layout: (p, f) = token p*F16+f.
                # sparse_gather's scan order (f*16+p) then yields an
                # interleaved — but consistent — permutation of the selected
                # set, which is all the FFN needs.
                z16 = gate_pool.tile([16, F16], dt.float32)
                nc.sync.dma_start(
                    z16[:], z_e_d[:].rearrange("q t -> (q t)").rearrange(
                        "(p f) -> p f", p=16))

                # dense zero-fill: issued on the sync queue right behind the
                # z readbacks; streams during the bisection, well before the
                # first output scatter needs it.
                for i in range(N // 128):
                    nc.sync.dma_start(dense_d[i * 128:(i + 1) * 128, :],
                                      zero_row[:])

                # ---- fp32 multi-probe bisection for the k-th largest ----
                probe_iota = gate_pool.tile([128, NPROBE], dt.int32)
                nc.gpsimd.iota(probe_iota[:], pattern=[[1, NPROBE]], base=1,
                               channel_multiplier=0)
                probe_f = gate_pool.tile([128, NPROBE], dt.float32)
                nc.vector.tensor_copy(probe_f[:], probe_iota[:])

                lo = gate_pool.tile([128, 1], dt.float32)
                hi = gate_pool.tile([128, 1], dt.float32)
                nc.vector.memset(lo[:], -BISECT_BOUND)
                nc.vector.memset(hi[:], BISECT_BOUND)
                kf = float(K)
                inv = 1.0 / (NPROBE + 1)
                for _ in range(BISECT_ROUNDS):
                    dl = small_pool.tile([128, 1], dt.float32, tag="dl")
                    nc.vector.tensor_tensor(dl[:], hi[:], lo[:],
                                            op=Alu.subtract)
                    nc.vector.tensor_scalar(dl[:], dl[:], inv, None,
                                            op0=Alu.mult)
                    mids = small_pool.tile([128, NPROBE], dt.float32,
                                           tag="mids")
                    nc.vector.tensor_scalar(mids[:], probe_f[:], dl[:, :1],
                                            lo[:, :1], op0=Alu.mult,
                                            op1=Alu.add)
                    part = small_pool.tile([128, NPROBE], dt.float32,
                                           tag="part")
                    for i in range(NPROBE):
                        cmpf = small_pool.tile([128, ZF], dt.float32,
                                               tag="cmpf")
                        nc.vector.tensor_scalar(cmpf[:], z_sb[:],
                                                mids[:, i:i + 1], None,
                                                op0=Alu.is_ge)
                        nc.vector.tensor_reduce(part[:, i:i + 1], cmpf[:],
                                                axis=mybir.AxisListType.X,
                                                op=Alu.add)
                    cnt_ps = psum1_pool.tile([128, NPROBE], dt.float32,
                                             tag="ps1")
                    nc.tensor.matmul(cnt_ps[:], ones128[:], part[:],
                                     start=True, stop=True)
                    cnt = small_pool.tile([128, NPROBE], dt.float32,
                                          tag="cnt")
                    nc.vector.tensor_copy(cnt[:], cnt_ps[:])
                    gemask = small_pool.tile([128, NPROBE], dt.uint8,
                                             tag="gemask")
                    ltmask = small_pool.tile([128, NPROBE], dt.uint8,
                                             tag="ltmask")
                    nc.vector.tensor_scalar(gemask[:], cnt[:], kf, None,
                                            op0=Alu.is_ge)
                    nc.vector.tensor_scalar(ltmask[:], cnt[:], kf, None,
                                            op0=Alu.is_lt)
                    # lo <- max(lo, max{mids[i] : count[i] >= K})
                    mlo = small_pool.tile([128, NPROBE], dt.float32,
                                          tag="mlo")
                    nc.vector.memset(mlo[:], -3e38)
                    nc.vector.copy_predicated(mlo[:], gemask[:], mids[:])
                    lomax = small_pool.tile([128, 1], dt.float32, tag="lomax")
                    nc.vector.tensor_reduce(lomax[:], mlo[:],
                                            axis=mybir.AxisListType.X,
                                            op=Alu.max)
                    nc.vector.tensor_tensor(lo[:], lo[:], lomax[:],
                                            op=Alu.max)
                    # hi <- min(hi, min{mids[i] : count[i] < K})
                    mhi = small_pool.tile([128, NPROBE], dt.float32,
                                          tag="mhi")
                    nc.vector.memset(mhi[:], 3e38)
                    nc.vector.copy_predicated(mhi[:], ltmask[:], mids[:])
                    himin = small_pool.tile([128, 1], dt.float32, tag="himin")
                    nc.vector.tensor_reduce(himin[:], mhi[:],
                                            axis=mybir.AxisListType.X,
                                            op=Alu.min)
                    nc.vector.tensor_tensor(hi[:], hi[:], himin[:],
                                            op=Alu.min)

                # ---- selection mask + sparse_gather compaction ----
                ids16 = gate_pool.tile([16, F16], dt.int32)
                nc.gpsimd.iota(ids16[:], pattern=[[1, F16]], base=0,
                               channel_multiplier=F16)
                idsf16 = gate_pool.tile([16, F16], dt.float32)
                nc.vector.tensor_copy(idsf16[:], ids16[:])

                mask16 = gate_pool.tile([16, F16], dt.uint8)
                nc.vector.tensor_scalar(mask16[:], z16[:], lo[0:16, 0:1],
                                        None, op0=Alu.is_ge)
                g16 = gate_pool.tile([16, F16], dt.float32)
                nc.scalar.activation(g16[:], z16[:], Act.Sigmoid)

                idm = gate_pool.tile([16, F16], dt.float32)
                nc.vector.memset(idm[:], -1.0)
                nc.vector.copy_predicated(idm[:], mask16[:], idsf16[:])
                gm = gate_pool.tile([16, F16], dt.float32)
                nc.vector.memset(gm[:], -1.0)
                nc.vector.copy_predicated(gm[:], mask16[:], g16[:])

                idc = gate_pool.tile([16, KOUT], dt.float32)
                nf1 = gate_pool.tile([1, 1], dt.uint32)
                nc.gpsimd.sparse_gather(idc[:], idm[:], num_found=nf1[:])
                gc = gate_pool.tile([16, KOUT], dt.float32)
                nf2 = gate_pool.tile([1, 1], dt.uint32)
                nc.gpsimd.sparse_gather(gc[:], gm[:], num_found=nf2[:])

                # compacted slot q lives at (q%16, q//16) in [16, K/16];
                # transpose -> [K/16=128, 16]: column s holds slots
                # q in {s, s+16, ...}: a valid subtile permutation.
                assert K // 16 == 128 and NCOLS == 16
                idT_ps = ptrans_pool.tile([128, 16], dt.float32, tag="pt")
                nc.tensor.transpose(idT_ps[:], idc[:, 0:K // 16], ident16[:])
                nc.vector.tensor_copy(ids_pp[:], idT_ps[:])
                gT_ps = ptrans_pool.tile([128, 16], dt.float32, tag="pt")
                nc.tensor.transpose(gT_ps[:], gc[:, 0:K // 16], ident16[:])
                nc.vector.tensor_copy(g_pp[:], gT_ps[:])

            # ================= FFN phase ================
            with (
                tc.tile_pool(name="ext", bufs=2) as ext_pool,
                tc.tile_pool(name="ex", bufs=1) as ex_pool,
                tc.tile_pool(name="hid", bufs=1) as hid_pool,
                tc.tile_pool(name="out", bufs=2) as out_pool,
            ):
                for g in range(NG):
                    # gather selected token rows (token-major)
                    ex_tok = ext_pool.tile([128, SUBS, H], dt.bfloat16,
                                           tag="ext")
                    for s in range(SUBS):
                        nc.gpsimd.indirect_dma_start(
                            out=ex_tok[:, s, :],
                            out_offset=None,
                            in_=x_bf[:],
                            in_offset=bass.IndirectOffsetOnAxis(
                                ap=ids_pp[:, g * SUBS + s:g * SUBS + s + 1],
                                axis=0),
                        )

                    # transpose to [h, tok] layout for the PE
                    ex_T = ex_pool.tile([128, HC, TOKG], dt.bfloat16,
                                        tag="ex")
                    for s in range(SUBS):
                        for ci in range(HC):
                            pt = ptrans_pool.tile([128, 128], dt.bfloat16,
                                                  tag="pt")
                            nc.tensor.transpose(
                                pt[:], ex_tok[:, s, ci * 128:(ci + 1) * 128],
                                ident_bf[:])
                            nc.vector.tensor_copy(
                                ex_T[:, ci, s * 128:(s + 1) * 128], pt[:])

                    hid_sb = hid_pool.tile([128, FC, TOKG], dt.bfloat16,
                                           tag="hid")
                    for fc in range(FC):
                        ps1 = psum1_pool.tile([128, TOKG], dt.float32,
                                              tag="ps1")
                        for ci in range(HC):
                            nc.tensor.matmul(
                                ps1[:], w1_sb[:, ci, fc * 128:(fc + 1) * 128],
                                ex_T[:, ci, :],
                                start=(ci == 0), stop=(ci == HC - 1))
                        nc.scalar.activation(hid_sb[:, fc, :], ps1[:], act,
                                             bias=b1_pp[:, fc:fc + 1])

                    for s in range(SUBS):
                        col = g * SUBS + s
                        pso = psum2_pool.tile([128, H], dt.float32, tag="ps2")
                        for hh in range(0, H, NSTEP):
                            for fc in range(FC):
                                nc.tensor.matmul(
                                    pso[:, hh:hh + NSTEP],
                                    hid_sb[:, fc, s * 128:(s + 1) * 128],
                                    w2_sb[:, fc, hh:hh + NSTEP],
                                    start=(fc == 0), stop=(fc == FC - 1))
                        out_bf = out_pool.tile([128, H], dt.bfloat16,
                                               tag="obf")
                        nc.vector.tensor_tensor(out_bf[:], pso[:],
                                                b2_bcast[:], op=Alu.add)
                        nc.vector.tensor_scalar(out_bf[:], out_bf[:],
                                                g_pp[:, col:col + 1], None,
                                                op0=Alu.mult)
                        nc.gpsimd.indirect_dma_start(
                            out=dense_d[:],
                            out_offset=bass.IndirectOffsetOnAxis(
                                ap=ids_pp[:, col:col + 1], axis=0),
                            in_=out_bf[:],
                            in_offset=None,
                        )

                # ---------------- combine ----------------
                nc.gpsimd.collective_compute(
                    "ReduceScatter", Alu.add, replica_groups=groups,
                    ins=[dense_d[:]], outs=[rs_out_d[:]],
                )
                nc.sync.dma_start(y_d[:], rs_out_d[:])

    _insert_library_loads(nc)
    _split_excess_waits(nc)
    return nc


# ---------------------------------------------------------------------------
# host-side sharding + execution
# ---------------------------------------------------------------------------

def make_in_maps(x, Wg, W1, b1, W2, b2, N=8192, H=1024):
    xt = np.ascontiguousarray(x.reshape(N, H).astype(np.float32))
    x_bf = xt.astype(ml_dtypes.bfloat16)
    P = N // N_CORES
    in_maps = []
    for c in range(N_CORES):
        shard = xt[c * P:(c + 1) * P, :]
        in_maps.append({
            "xT_s": np.ascontiguousarray(shard.T),
            "x_bf": x_bf,
            "Wg": np.ascontiguousarray(Wg.astype(np.float32)),
            "W1": np.ascontiguousarray(W1[c].astype(ml_dtypes.bfloat16)),
            "W2": np.ascontiguousarray(W2[c].astype(ml_dtypes.bfloat16)),
            "b1": np.ascontiguousarray(b1[c].reshape(1, -1).astype(np.float32)),
            "b2": np.ascontiguousarray(b2[c].reshape(1, -1).astype(np.float32)),
        })
    return in_maps


_NC_CACHE = {}


def kernel(x, Wg, W1, b1, W2, b2):
    x = np.asarray(x)
    B, L, H = x.shape
    N = B * L
    FF = W1.shape[2]
    key = (N, H, FF)
    if key not in _NC_CACHE:
        _NC_CACHE[key] = build_moe_nc(N=N, H=H, FF=FF)
    nc = _NC_CACHE[key]
    in_maps = make_in_maps(np.asarray(x), np.asarray(Wg), np.asarray(W1),
                           np.asarray(b1), np.asarray(W2), np.asarray(b2),
                           N=N, H=H)
    from concourse.bass_utils import run_bass_kernel_spmd
    res = run_bass_kernel_spmd(nc, in_maps, core_ids=list(range(N_CORES)),
                               trace=False)
    out = np.concatenate(
        [np.asarray(res.results[c]["y"]).astype(np.float32)
         for c in range(N_CORES)], axis=0)
    return out.reshape(B, L, H)


# revision 28
# speedup vs baseline: 1.1540x; 1.1540x over previous
"""Expert-choice MoE kernel for 8 Trainium2 NeuronCores (Bass/Tile).

Distribution: expert-parallel, one expert per core.
  - gate: each core computes fp32 scores z = x_shard @ Wg for its 1/8 token
    shard, AllToAll -> full (E, N) scores on every core (core c gets expert c).
  - top-k (k=2048 of N=8192) per expert: exact threshold via fp32 bisection
    (34 steps) on the global count, then index compaction with the gpsimd
    sparse_gather ucode op (SBUF-only stream compaction) + one PE transpose.
  - dispatch: indirect-DMA gather of the 2048 selected token rows (bf16).
  - expert FFN in bf16 (fp32 accumulation), erf-Gelu on the scalar engine,
    fp32 gate multiply on the way out, cast to bf16.
  - combine: scatter rows into a per-core dense bf16 (N, H) buffer,
    ReduceScatter (add) across the 8 cores -> each core owns one token shard
    of the output (bf16, upcast on host).
"""

import sys

for _p in ("/opt/trn_rl_repo",):
    if _p not in sys.path:
        sys.path.insert(0, _p)

import numpy as np
import ml_dtypes

import concourse.bass as bass
import concourse.mybir as mybir
import concourse.tile as tile
import bass_rust

# ---------------------------------------------------------------------------
# Patch: this walrus build rejects >1 sync-wait on the SP Drain that
# TileContext emits at kernel exit. Split the global-clock waits across
# several drains (1 wait each).
# ---------------------------------------------------------------------------
from concourse.vector_clock import ScopedClock

_MAX_DRAIN_WAITS = 1


def _patched_drain_and_barrier(self, tick_clock, wait_clock):
    nc = self.nc
    probe = nc.sync.drain()
    wait_clock.add_sem_waits(probe.ins, ScopedClock({None: tick_clock.global_clock}))
    si = probe.ins.sync_info
    waits = list(si.on_wait or []) if si is not None else []
    if len(waits) > _MAX_DRAIN_WAITS:
        probe.ins.sync_info = mybir.SyncInfo(
            on_wait=waits[:_MAX_DRAIN_WAITS],
            on_update=list(si.on_update or []),
        )
        for i in range(_MAX_DRAIN_WAITS, len(waits), _MAX_DRAIN_WAITS):
            extra = nc.sync.drain()
            extra.ins.sync_info = mybir.SyncInfo(
                on_wait=waits[i : i + _MAX_DRAIN_WAITS], on_update=[]
            )
    nc.all_engine_barrier()
    assert self.sems is not None
    popped = nc._tile_sem_poison_stack.pop()
    assert popped is self._sem_poison
    nc.clear_and_free_semaphores(list(self.sems.allocated().values()))
    nc.all_engine_barrier()


tile.TileContext._drain_and_barrier = _patched_drain_and_barrier

_WSPLIT_LIMIT = 1
_wsplit_ctr = [0]


def _split_excess_waits(nc, limit=_WSPLIT_LIMIT):
    """This walrus build encodes at most `limit` sync-wait commands per
    instruction; hoist excess waits onto same-engine Drain instructions
    inserted immediately before (per-engine streams execute in order)."""
    f = nc.m.functions[0]
    for b in f.blocks:
        insts = b.instructions
        out = []
        changed = False
        for inst in insts:
            si = getattr(inst, "sync_info", None)
            waits = list(si.on_wait or []) if si is not None else []
            eng = getattr(inst, "engine", None)
            if len(waits) > limit and eng is not None and \
                    eng != mybir.EngineType.Unassigned:
                keep = waits[-limit:]
                extra = waits[:-limit]
                for i in range(0, len(extra), limit):
                    d = mybir.InstDrain(
                        name=f"WSPLIT-{_wsplit_ctr[0]}", ins=[], outs=[])
                    _wsplit_ctr[0] += 1
                    d.engine = eng
                    d.sync_info = mybir.SyncInfo(
                        on_wait=extra[i:i + limit], on_update=[])
                    out.append(d)
                    nc.register_instruction(d, overwrite=True)
                inst.sync_info = mybir.SyncInfo(
                    on_wait=keep, on_update=list(si.on_update or []))
                changed = True
            out.append(inst)
        if changed:
            b.instructions = out


def _insert_library_loads(nc):
    from concourse.library_config import all_libraries, standard
    m = {}
    for lib in all_libraries:
        for t in lib.instructions:
            m[t] = m.get(t, 0) | (1 << lib.index)
    bass_rust.insert_library_loads(nc, m, len(all_libraries), standard.index)
    bass_rust.codegen_inst_isa_subclasses(nc)


dt = mybir.dt
Alu = mybir.AluOpType
Act = mybir.ActivationFunctionType

N_CORES = 8

# problem dims (full size; can be shrunk for simulation)
FULL = dict(N=8192, H=1024, FF=4096, E=8, K=2048)


def build_moe_nc(N=8192, H=1024, FF=4096, E=8, K=2048, TOKG=512, act=None):
    """Build the SPMD Bass program (same program on all 8 cores)."""
    assert E == N_CORES
    P = N // N_CORES          # tokens per shard
    HC = H // 128             # h chunks
    FC = FF // 128            # ff chunks
    NG = K // TOKG            # token groups
    SUBS = TOKG // 128        # 128-token subtiles per group
    NCOLS = K // 128          # total 128-token subtiles
    ZF = N // 128             # free size of the [128, ZF] score layout
    F16 = N // 16             # free size of the [16, F16] score layout
    KOUT = K // 16 + 32       # sparse_gather output cols (slack for ties)
    assert K % TOKG == 0 and TOKG % 128 == 0 and P % 128 == 0
    assert K == NCOLS * 128 and NCOLS <= 128
    if act is None:
        act = Act.Gelu
    NSTEP = min(512, H)
    # multi-probe bisection: NPROBE interior probes shrink the bracket by
    # (NPROBE+1)x per round; 12 rounds from +-16 -> 4.7e-10 < fp32 ulp of
    # the threshold (~3e-8), so the final count is exactly K (absent ties).
    NPROBE = 7
    BISECT_ROUNDS = 12
    BISECT_BOUND = 16.0

    nc = bass.Bass(num_devices=N_CORES)

    # ---- I/O ----
    xT_s = nc.dram_tensor("xT_s", [H, P], dt.float32, kind="ExternalInput")
    x_bfs = nc.dram_tensor("x_bfs", [P, H], dt.bfloat16, kind="ExternalInput")
    Wg_d = nc.dram_tensor("Wg", [H, E], dt.float32, kind="ExternalInput")
    W1_d = nc.dram_tensor("W1", [H, FF], dt.bfloat16, kind="ExternalInput")
    W2_d = nc.dram_tensor("W2", [FF, H], dt.bfloat16, kind="ExternalInput")
    b1_d = nc.dram_tensor("b1", [1, FF], dt.float32, kind="ExternalInput")
    b2_d = nc.dram_tensor("b2", [1, H], dt.float32, kind="ExternalInput")
    y_d = nc.dram_tensor("y", [P, H], dt.bfloat16, kind="ExternalOutput")

    # ---- internal DRAM ----
    z_loc_d = nc.dram_tensor("z_loc", [E, P], dt.float32)
    z_e_d = nc.dram_tensor("z_e", [N_CORES, P], dt.float32)
    xs_stage_d = nc.dram_tensor("xs_stage", [P, H], dt.bfloat16)
    xg_d = nc.dram_tensor("xg", [N, H], dt.bfloat16)
    dense_d = nc.dram_tensor("dense", [N, H], dt.bfloat16)
    rs_out_d = nc.dram_tensor("rs_out", [P, H], dt.bfloat16)

    groups = [list(range(N_CORES))]

    with tile.TileContext(nc) as tc:
        with (
            tc.tile_pool(name="const", bufs=1) as const_pool,
            tc.tile_pool(name="w", bufs=1) as w_pool,
            tc.tile_pool(name="psum1", bufs=2, space="PSUM") as psum1_pool,
            tc.tile_pool(name="psum2", bufs=2, space="PSUM") as psum2_pool,
            tc.tile_pool(name="ptrans", bufs=2, space="PSUM") as ptrans_pool,
        ):
            # ---------------- persistent constants ----------------
            ones1 = const_pool.tile([1, 128], dt.float32)
            nc.vector.memset(ones1[:], 1.0)

            # f - p iota: identity masks
            fmp = const_pool.tile([128, 128], dt.int32)
            nc.gpsimd.iota(fmp[:], pattern=[[1, 128]], base=0,
                           channel_multiplier=-1)
            fmp_f = const_pool.tile([128, 128], dt.float32)
            nc.vector.tensor_copy(fmp_f[:], fmp[:])
            ident_bf = const_pool.tile([128, 128], dt.bfloat16)
            nc.vector.tensor_scalar(ident_bf[:], fmp_f[:], 0.0, None,
                                    op0=Alu.is_equal)
            ident16 = const_pool.tile([16, 16], dt.float32)
            nc.vector.tensor_scalar(ident16[:], fmp_f[0:16, 0:16], 0.0, None,
                                    op0=Alu.is_equal)
            ones128 = const_pool.tile([128, 128], dt.float32)
            nc.vector.memset(ones128[:], 1.0)

            # b2 broadcast [128, H] (constant along tokens)
            b2_sb = const_pool.tile([1, H], dt.float32)
            nc.sync.dma_start(b2_sb[:], b2_d[:])
            b2_ps = psum2_pool.tile([128, H], dt.float32, tag="ps2")
            for hh in range(0, H, NSTEP):
                nc.tensor.matmul(b2_ps[:, hh:hh + NSTEP], ones1[:],
                                 b2_sb[:, hh:hh + NSTEP], start=True, stop=True)
            b2_bcast = const_pool.tile([128, H], dt.float32)
            nc.vector.tensor_copy(b2_bcast[:], b2_ps[:])

            # b1 per-partition [128, FC]
            b1_pp = const_pool.tile([128, FC], dt.float32)
            nc.sync.dma_start(
                b1_pp[:], b1_d[:].rearrange("o (c p) -> (o p) c", p=128))

            zero_row = const_pool.tile([128, H], dt.bfloat16)
            nc.vector.memset(zero_row[:], 0.0)

            # persistent routing outputs (filled by the gate phase)
            ids_pp = const_pool.tile([128, NCOLS], dt.int32)
            g_pp = const_pool.tile([128, NCOLS], dt.float32)

            # ================= gate phase (scoped pool) ================
            with (
                tc.tile_pool(name="gate", bufs=1) as gate_pool,
                tc.tile_pool(name="small", bufs=2) as small_pool,
            ):
                # PE warm-up: a few dependency-free matmuls ramp the tensor
                # engine out of its low p-state during the entry barrier, so
                # the latency-critical gate matmul runs at speed.
                warm_src = gate_pool.tile([128, 512], dt.float32)
                nc.vector.memset(warm_src[:], 0.0)
                for wi in range(8):
                    warm_ps = psum1_pool.tile([128, 512], dt.float32,
                                              tag="ps1")
                    nc.tensor.matmul(warm_ps[:], ones128[:], warm_src[:],
                                     start=True, stop=True)

                # x shard staging heads the sync queue: the AllGather is the
                # longest pole of the routing phase and must start first.
                nc.sync.dma_start(xs_stage_d[:], x_bfs[:])
                # the x AllGather is issued first on the CC queue; the tiny
                # AllToAll issued later rides right behind it.
                nc.gpsimd.collective_compute(
                    "AllGather", Alu.bypass, replica_groups=groups,
                    ins=[xs_stage_d[:]], outs=[xg_d[:]],
                )

                # gate inputs next: they head the z-score critical path
                xT_sb = gate_pool.tile([128, HC, P], dt.float32)
                for ci in range(HC):
                    nc.sync.dma_start(
                        xT_sb[:, ci, :], xT_s[ci * 128:(ci + 1) * 128, :])
                wg_sb = gate_pool.tile([128, HC, E], dt.float32)
                nc.sync.dma_start(
                    wg_sb[:], Wg_d[:].rearrange("(c p) e -> p c e", p=128))

                z_sb_loc = gate_pool.tile([E, P], dt.float32)
                for t0 in range(0, P, 512):
                    zw = min(512, P - t0)
                    z_ps = psum1_pool.tile([E, 512], dt.float32, tag="ps1")
                    for ci in range(HC):
                        nc.tensor.matmul(z_ps[:, :zw], wg_sb[:, ci, :],
                                         xT_sb[:, ci, t0:t0 + zw],
                                         start=(ci == 0), stop=(ci == HC - 1))
                    nc.vector.tensor_copy(z_sb_loc[:, t0:t0 + zw],
                                          z_ps[:, :zw])
                nc.sync.dma_start(z_loc_d[:], z_sb_loc[:])

                # core c receives every shard's scores for expert c
                nc.gpsimd.collective_compute(
                    "AllToAll", Alu.bypass, replica_groups=groups,
                    ins=[z_loc_d[:]], outs=[z_e_d[:]],
                )

                z_sb = gate_pool.tile([128, ZF], dt.float32)
                nc.sync.dma_start(
                    z_sb[:], z_e_d[:].rearrange("q t -> (q t)").rearrange(
                        "(p f) -> p f", p=128))
                # contiguous per-partition layout: (p, f) = token p*F16+f.
                # sparse_gather's scan order (f*16+p) then yields an
                # interleaved — but consistent — permutation of the selected
                # set, which is all the FFN needs.
                z16 = gate_pool.tile([16, F16], dt.float32)
                nc.sync.dma_start(
                    z16[:], z_e_d[:].rearrange("q t -> (q t)").rearrange(
                        "(p f) -> p f", p=16))

                # ---- bulk DMAs: issued on the sync queue BEHIND the z
                # readbacks, so the sync engine stalls on the AllToAll before
                # releasing them — they never contend with the small
                # latency-critical collective, and they stream during the
                # bisection / AllGather, done before the FFN needs them. ----
                w1_sb = w_pool.tile([128, HC, FF], dt.bfloat16)
                for ci in range(HC):
                    nc.sync.dma_start(
                        w1_sb[:, ci, :], W1_d[ci * 128:(ci + 1) * 128, :])
                w2_sb = w_pool.tile([128, FC, H], dt.bfloat16)
                for fc in range(FC):
                    nc.sync.dma_start(
                        w2_sb[:, fc, :], W2_d[fc * 128:(fc + 1) * 128, :])
                for i in range(N // 128):
                    nc.sync.dma_start(dense_d[i * 128:(i + 1) * 128, :],
                                      zero_row[:])

                # ---- fp32 multi-probe bisection for the k-th largest ----
                probe_iota = gate_pool.tile([128, NPROBE], dt.int32)
                nc.gpsimd.iota(probe_iota[:], pattern=[[1, NPROBE]], base=1,
                               channel_multiplier=0)
                probe_f = gate_pool.tile([128, NPROBE], dt.float32)
                nc.vector.tensor_copy(probe_f[:], probe_iota[:])

                lo = gate_pool.tile([128, 1], dt.float32)
                hi = gate_pool.tile([128, 1], dt.float32)
                nc.vector.memset(lo[:], -BISECT_BOUND)
                nc.vector.memset(hi[:], BISECT_BOUND)
                kf = float(K)
                inv = 1.0 / (NPROBE + 1)
                for _ in range(BISECT_ROUNDS):
                    dl = small_pool.tile([128, 1], dt.float32, tag="dl")
                    nc.vector.tensor_tensor(dl[:], hi[:], lo[:],
                                            op=Alu.subtract)
                    nc.vector.tensor_scalar(dl[:], dl[:], inv, None,
                                            op0=Alu.mult)
                    mids = small_pool.tile([128, NPROBE], dt.float32,
                                           tag="mids")
                    nc.vector.tensor_scalar(mids[:], probe_f[:], dl[:, :1],
                                            lo[:, :1], op0=Alu.mult,
                                            op1=Alu.add)
                    part = small_pool.tile([128, NPROBE], dt.float32,
                                           tag="part")
                    for i in range(NPROBE):
                        cmpf = small_pool.tile([128, ZF], dt.float32,
                                               tag="cmpf")
                        nc.vector.tensor_scalar(cmpf[:], z_sb[:],
                                                mids[:, i:i + 1], None,
                                                op0=Alu.is_ge)
                        nc.vector.tensor_reduce(part[:, i:i + 1], cmpf[:],
                                                axis=mybir.AxisListType.X,
                                                op=Alu.add)
                    cnt_ps = psum1_pool.tile([128, NPROBE], dt.float32,
                                             tag="ps1")
                    nc.tensor.matmul(cnt_ps[:], ones128[:], part[:],
                                     start=True, stop=True)
                    cnt = small_pool.tile([128, NPROBE], dt.float32,
                                          tag="cnt")
                    nc.vector.tensor_copy(cnt[:], cnt_ps[:])
                    gemask = small_pool.tile([128, NPROBE], dt.uint8,
                                             tag="gemask")
                    ltmask = small_pool.tile([128, NPROBE], dt.uint8,
                                             tag="ltmask")
                    nc.vector.tensor_scalar(gemask[:], cnt[:], kf, None,
                                            op0=Alu.is_ge)
                    nc.vector.tensor_scalar(ltmask[:], cnt[:], kf, None,
                                            op0=Alu.is_lt)
                    # lo <- max(lo, max{mids[i] : count[i] >= K})
                    mlo = small_pool.tile([128, NPROBE], dt.float32,
                                          tag="mlo")
                    nc.vector.memset(mlo[:], -3e38)
                    nc.vector.copy_predicated(mlo[:], gemask[:], mids[:])
                    lomax = small_pool.tile([128, 1], dt.float32, tag="lomax")
                    nc.vector.tensor_reduce(lomax[:], mlo[:],
                                            axis=mybir.AxisListType.X,
                                            op=Alu.max)
                    nc.vector.tensor_tensor(lo[:], lo[:], lomax[:],
                                            op=Alu.max)
                    # hi <- min(hi, min{mids[i] : count[i] < K})
                    mhi = small_pool.tile([128, NPROBE], dt.float32,
                                          tag="mhi")
                    nc.vector.memset(mhi[:], 3e38)
                    nc.vector.copy_predicated(mhi[:], ltmask[:], mids[:])
                    himin = small_pool.tile([128, 1], dt.float32, tag="himin")
                    nc.vector.tensor_reduce(himin[:], mhi[:],
                                            axis=mybir.AxisListType.X,
                                            op=Alu.min)
                    nc.vector.tensor_tensor(hi[:], hi[:], himin[:],
                                            op=Alu.min)

                # ---- selection mask + sparse_gather compaction ----
                ids16 = gate_pool.tile([16, F16], dt.int32)
                nc.gpsimd.iota(ids16[:], pattern=[[1, F16]], base=0,
                               channel_multiplier=F16)
                idsf16 = gate_pool.tile([16, F16], dt.float32)
                nc.vector.tensor_copy(idsf16[:], ids16[:])

                mask16 = gate_pool.tile([16, F16], dt.uint8)
                nc.vector.tensor_scalar(mask16[:], z16[:], lo[0:16, 0:1],
                                        None, op0=Alu.is_ge)
                g16 = gate_pool.tile([16, F16], dt.float32)
                nc.scalar.activation(g16[:], z16[:], Act.Sigmoid)

                idm = gate_pool.tile([16, F16], dt.float32)
                nc.vector.memset(idm[:], -1.0)
                nc.vector.copy_predicated(idm[:], mask16[:], idsf16[:])
                gm = gate_pool.tile([16, F16], dt.float32)
                nc.vector.memset(gm[:], -1.0)
                nc.vector.copy_predicated(gm[:], mask16[:], g16[:])

                idc = gate_pool.tile([16, KOUT], dt.float32)
                nf1 = gate_pool.tile([1, 1], dt.uint32)
                nc.gpsimd.sparse_gather(idc[:], idm[:], num_found=nf1[:])
                gc = gate_pool.tile([16, KOUT], dt.float32)
                nf2 = gate_pool.tile([1, 1], dt.uint32)
                nc.gpsimd.sparse_gather(gc[:], gm[:], num_found=nf2[:])

                # compacted slot q lives at (q%16, q//16) in [16, K/16];
                # transpose -> [K/16=128, 16]: column s holds slots
                # q in {s, s+16, ...}: a valid subtile permutation.
                assert K // 16 == 128 and NCOLS == 16
                idT_ps = ptrans_pool.tile([128, 16], dt.float32, tag="pt")
                nc.tensor.transpose(idT_ps[:], idc[:, 0:K // 16], ident16[:])
                nc.vector.tensor_copy(ids_pp[:], idT_ps[:])
                gT_ps = ptrans_pool.tile([128, 16], dt.float32, tag="pt")
                nc.tensor.transpose(gT_ps[:], gc[:, 0:K // 16], ident16[:])
                nc.vector.tensor_copy(g_pp[:], gT_ps[:])

            # ================= FFN phase ================
            with (
                tc.tile_pool(name="ext", bufs=2) as ext_pool,
                tc.tile_pool(name="ex", bufs=1) as ex_pool,
                tc.tile_pool(name="hid", bufs=1) as hid_pool,
                tc.tile_pool(name="out", bufs=2) as out_pool,
            ):
                for g in range(NG):
                    # gather selected token rows (token-major)
                    ex_tok = ext_pool.tile([128, SUBS, H], dt.bfloat16,
                                           tag="ext")
                    for s in range(SUBS):
                        nc.gpsimd.indirect_dma_start(
                            out=ex_tok[:, s, :],
                            out_offset=None,
                            in_=xg_d[:],
                            in_offset=bass.IndirectOffsetOnAxis(
                                ap=ids_pp[:, g * SUBS + s:g * SUBS + s + 1],
                                axis=0),
                        )

                    # transpose to [h, tok] layout for the PE
                    ex_T = ex_pool.tile([128, HC, TOKG], dt.bfloat16,
                                        tag="ex")
                    for s in range(SUBS):
                        for ci in range(HC):
                            pt = ptrans_pool.tile([128, 128], dt.bfloat16,
                                                  tag="pt")
                            nc.tensor.transpose(
                                pt[:], ex_tok[:, s, ci * 128:(ci + 1) * 128],
                                ident_bf[:])
                            nc.vector.tensor_copy(
                                ex_T[:, ci, s * 128:(s + 1) * 128], pt[:])

                    hid_sb = hid_pool.tile([128, FC, TOKG], dt.bfloat16,
                                           tag="hid")
                    for fc in range(FC):
                        ps1 = psum1_pool.tile([128, TOKG], dt.float32,
                                              tag="ps1")
                        for ci in range(HC):
                            nc.tensor.matmul(
                                ps1[:], w1_sb[:, ci, fc * 128:(fc + 1) * 128],
                                ex_T[:, ci, :],
                                start=(ci == 0), stop=(ci == HC - 1))
                        nc.scalar.activation(hid_sb[:, fc, :], ps1[:], act,
                                             bias=b1_pp[:, fc:fc + 1])

                    for s in range(SUBS):
                        col = g * SUBS + s
                        pso = psum2_pool.tile([128, H], dt.float32, tag="ps2")
                        for hh in range(0, H, NSTEP):
                            for fc in range(FC):
                                nc.tensor.matmul(
                                    pso[:, hh:hh + NSTEP],
                                    hid_sb[:, fc, s * 128:(s + 1) * 128],
                                    w2_sb[:, fc, hh:hh + NSTEP],
                                    start=(fc == 0), stop=(fc == FC - 1))
                        out_bf = out_pool.tile([128, H], dt.bfloat16,
                                               tag="obf")
                        nc.vector.tensor_tensor(out_bf[:], pso[:],
                                                b2_bcast[:], op=Alu.add)
                        nc.vector.tensor_scalar(out_bf[:], out_bf[:],
                                                g_pp[:, col:col + 1], None,
                                                op0=Alu.mult)
                        nc.gpsimd.indirect_dma_start(
                            out=dense_d[:],
                            out_offset=bass.IndirectOffsetOnAxis(
                                ap=ids_pp[:, col:col + 1], axis=0),
                            in_=out_bf[:],
                            in_offset=None,
                        )

                # ---------------- combine ----------------
                nc.gpsimd.collective_compute(
                    "ReduceScatter", Alu.add, replica_groups=groups,
                    ins=[dense_d[:]], outs=[rs_out_d[:]],
                )
                nc.sync.dma_start(y_d[:], rs_out_d[:])

    _insert_library_loads(nc)
    _split_excess_waits(nc)
    return nc


# ---------------------------------------------------------------------------
# host-side sharding + execution
# ---------------------------------------------------------------------------

def make_in_maps(x, Wg, W1, b1, W2, b2, N=8192, H=1024):
    xt = np.ascontiguousarray(x.reshape(N, H).astype(np.float32))
    P = N // N_CORES
    in_maps = []
    for c in range(N_CORES):
        shard = xt[c * P:(c + 1) * P, :]
        in_maps.append({
            "xT_s": np.ascontiguousarray(shard.T),
            "x_bfs": np.ascontiguousarray(shard.astype(ml_dtypes.bfloat16)),
            "Wg": np.ascontiguousarray(Wg.astype(np.float32)),
            "W1": np.ascontiguousarray(W1[c].astype(ml_dtypes.bfloat16)),
            "W2": np.ascontiguousarray(W2[c].astype(ml_dtypes.bfloat16)),
            "b1": np.ascontiguousarray(b1[c].reshape(1, -1).astype(np.float32)),
            "b2": np.ascontiguousarray(b2[c].reshape(1, -1).astype(np.float32)),
        })
    return in_maps


_NC_CACHE = {}


def kernel(x, Wg, W1, b1, W2, b2):
    x = np.asarray(x)
    B, L, H = x.shape
    N = B * L
    FF = W1.shape[2]
    key = (N, H, FF)
    if key not in _NC_CACHE:
        _NC_CACHE[key] = build_moe_nc(N=N, H=H, FF=FF)
    nc = _NC_CACHE[key]
    in_maps = make_in_maps(np.asarray(x), np.asarray(Wg), np.asarray(W1),
                           np.asarray(b1), np.asarray(W2), np.asarray(b2),
                           N=N, H=H)
    from concourse.bass_utils import run_bass_kernel_spmd
    res = run_bass_kernel_spmd(nc, in_maps, core_ids=list(range(N_CORES)),
                               trace=False)
    out = np.concatenate(
        [np.asarray(res.results[c]["y"]).astype(np.float32)
         for c in range(N_CORES)], axis=0)
    return out.reshape(B, L, H)


# revision 36
# speedup vs baseline: 10.1254x; 8.7739x over previous
"""Expert-choice MoE kernel for 8 Trainium2 NeuronCores (Bass/Tile).

Distribution: expert-parallel, one expert per core.
  - gate: each core computes fp32 scores z = x_shard @ Wg for its 1/8 token
    shard, AllToAll -> full (E, N) scores on every core (core c gets expert c).
  - top-k (k=2048 of N=8192) per expert: exact threshold via fp32 bisection
    (34 steps) on the global count, then index compaction with the gpsimd
    sparse_gather ucode op (SBUF-only stream compaction) + one PE transpose.
  - dispatch: indirect-DMA gather of the 2048 selected token rows (bf16).
  - expert FFN in bf16 (fp32 accumulation), erf-Gelu on the scalar engine,
    fp32 gate multiply on the way out, cast to bf16.
  - combine: scatter rows into a per-core dense bf16 (N, H) buffer,
    ReduceScatter (add) across the 8 cores -> each core owns one token shard
    of the output (bf16, upcast on host).
"""

import sys

for _p in ("/opt/trn_rl_repo",):
    if _p not in sys.path:
        sys.path.insert(0, _p)

import numpy as np
import ml_dtypes

import concourse.bass as bass
import concourse.mybir as mybir
import concourse.tile as tile
import bass_rust

# ---------------------------------------------------------------------------
# Patch: this walrus build rejects >1 sync-wait on the SP Drain that
# TileContext emits at kernel exit. Split the global-clock waits across
# several drains (1 wait each).
# ---------------------------------------------------------------------------
from concourse.vector_clock import ScopedClock

_MAX_DRAIN_WAITS = 1


def _patched_drain_and_barrier(self, tick_clock, wait_clock):
    nc = self.nc
    probe = nc.sync.drain()
    wait_clock.add_sem_waits(probe.ins, ScopedClock({None: tick_clock.global_clock}))
    si = probe.ins.sync_info
    waits = list(si.on_wait or []) if si is not None else []
    if len(waits) > _MAX_DRAIN_WAITS:
        probe.ins.sync_info = mybir.SyncInfo(
            on_wait=waits[:_MAX_DRAIN_WAITS],
            on_update=list(si.on_update or []),
        )
        for i in range(_MAX_DRAIN_WAITS, len(waits), _MAX_DRAIN_WAITS):
            extra = nc.sync.drain()
            extra.ins.sync_info = mybir.SyncInfo(
                on_wait=waits[i : i + _MAX_DRAIN_WAITS], on_update=[]
            )
    nc.all_engine_barrier()
    assert self.sems is not None
    popped = nc._tile_sem_poison_stack.pop()
    assert popped is self._sem_poison
    nc.clear_and_free_semaphores(list(self.sems.allocated().values()))
    nc.all_engine_barrier()


tile.TileContext._drain_and_barrier = _patched_drain_and_barrier

_WSPLIT_LIMIT = 1
_wsplit_ctr = [0]


def _split_excess_waits(nc, limit=_WSPLIT_LIMIT):
    """This walrus build encodes at most `limit` sync-wait commands per
    instruction; hoist excess waits onto same-engine Drain instructions
    inserted immediately before (per-engine streams execute in order)."""
    f = nc.m.functions[0]
    for b in f.blocks:
        insts = b.instructions
        out = []
        changed = False
        for inst in insts:
            si = getattr(inst, "sync_info", None)
            waits = list(si.on_wait or []) if si is not None else []
            eng = getattr(inst, "engine", None)
            if len(waits) > limit and eng is not None and \
                    eng != mybir.EngineType.Unassigned:
                keep = waits[-limit:]
                extra = waits[:-limit]
                for i in range(0, len(extra), limit):
                    d = mybir.InstDrain(
                        name=f"WSPLIT-{_wsplit_ctr[0]}", ins=[], outs=[])
                    _wsplit_ctr[0] += 1
                    d.engine = eng
                    d.sync_info = mybir.SyncInfo(
                        on_wait=extra[i:i + limit], on_update=[])
                    out.append(d)
                    nc.register_instruction(d, overwrite=True)
                inst.sync_info = mybir.SyncInfo(
                    on_wait=keep, on_update=list(si.on_update or []))
                changed = True
            out.append(inst)
        if changed:
            b.instructions = out


def _insert_library_loads(nc):
    from concourse.library_config import all_libraries, standard
    m = {}
    for lib in all_libraries:
        for t in lib.instructions:
            m[t] = m.get(t, 0) | (1 << lib.index)
    bass_rust.insert_library_loads(nc, m, len(all_libraries), standard.index)
    bass_rust.codegen_inst_isa_subclasses(nc)


dt = mybir.dt
Alu = mybir.AluOpType
Act = mybir.ActivationFunctionType

N_CORES = 8

# problem dims (full size; can be shrunk for simulation)
FULL = dict(N=8192, H=1024, FF=4096, E=8, K=2048)


def build_moe_nc(N=8192, H=1024, FF=4096, E=8, K=2048, TOKG=512, act=None):
    """Build the SPMD Bass program (same program on all 8 cores)."""
    assert E == N_CORES
    P = N // N_CORES          # tokens per shard
    HC = H // 128             # h chunks
    FC = FF // 128            # ff chunks
    NG = K // TOKG            # token groups
    SUBS = TOKG // 128        # 128-token subtiles per group
    NCOLS = K // 128          # total 128-token subtiles
    ZF = N // 128             # free size of the [128, ZF] score layout
    F16 = N // 16             # free size of the [16, F16] score layout
    KOUT = K // 16 + 32       # sparse_gather output cols (slack for ties)
    assert K % TOKG == 0 and TOKG % 128 == 0 and P % 128 == 0
    assert K == NCOLS * 128 and NCOLS <= 128
    if act is None:
        act = Act.Gelu
    NSTEP = min(512, H)
    # multi-probe bisection: NPROBE interior probes shrink the bracket by
    # (NPROBE+1)x per round; 12 rounds from +-16 -> 4.7e-10 < fp32 ulp of
    # the threshold (~3e-8), so the final count is exactly K (absent ties).
    NPROBE = 7
    BISECT_ROUNDS = 12
    BISECT_BOUND = 16.0

    nc = bass.Bass(num_devices=N_CORES)

    # ---- I/O ----
    xT_s = nc.dram_tensor("xT_s", [H, P], dt.float32, kind="ExternalInput")
    x_bf = nc.dram_tensor("x_bf", [N, H], dt.bfloat16, kind="ExternalInput")
    Wg_d = nc.dram_tensor("Wg", [H, E], dt.float32, kind="ExternalInput")
    W1_d = nc.dram_tensor("W1", [H, FF], dt.bfloat16, kind="ExternalInput")
    W2_d = nc.dram_tensor("W2", [FF, H], dt.bfloat16, kind="ExternalInput")
    b1_d = nc.dram_tensor("b1", [1, FF], dt.float32, kind="ExternalInput")
    b2_d = nc.dram_tensor("b2", [1, H], dt.float32, kind="ExternalInput")
    y_d = nc.dram_tensor("y", [P, H], dt.bfloat16, kind="ExternalOutput")

    # ---- internal DRAM ----
    z_loc_d = nc.dram_tensor("z_loc", [E, P], dt.float32)
    z_e_d = nc.dram_tensor("z_e", [N_CORES, P], dt.float32)
    dense_d = nc.dram_tensor("dense", [N, H], dt.bfloat16)
    rs_out_d = nc.dram_tensor("rs_out", [P, H], dt.bfloat16)

    groups = [list(range(N_CORES))]

    with tile.TileContext(nc) as tc:
        with (
            tc.tile_pool(name="const", bufs=1) as const_pool,
            tc.tile_pool(name="w", bufs=1) as w_pool,
            tc.tile_pool(name="psum1", bufs=2, space="PSUM") as psum1_pool,
            tc.tile_pool(name="psum2", bufs=2, space="PSUM") as psum2_pool,
            tc.tile_pool(name="ptrans", bufs=2, space="PSUM") as ptrans_pool,
        ):
            # ---------------- persistent constants ----------------
            ones1 = const_pool.tile([1, 128], dt.float32)
            nc.vector.memset(ones1[:], 1.0)

            # f - p iota: identity masks
            fmp = const_pool.tile([128, 128], dt.int32)
            nc.gpsimd.iota(fmp[:], pattern=[[1, 128]], base=0,
                           channel_multiplier=-1)
            fmp_f = const_pool.tile([128, 128], dt.float32)
            nc.vector.tensor_copy(fmp_f[:], fmp[:])
            ident_bf = const_pool.tile([128, 128], dt.bfloat16)
            nc.vector.tensor_scalar(ident_bf[:], fmp_f[:], 0.0, None,
                                    op0=Alu.is_equal)
            ident16 = const_pool.tile([16, 16], dt.float32)
            nc.vector.tensor_scalar(ident16[:], fmp_f[0:16, 0:16], 0.0, None,
                                    op0=Alu.is_equal)
            ones128 = const_pool.tile([128, 128], dt.float32)
            nc.vector.memset(ones128[:], 1.0)

            # b2 broadcast [128, H] (constant along tokens)
            b2_sb = const_pool.tile([1, H], dt.float32)
            nc.sync.dma_start(b2_sb[:], b2_d[:])
            b2_ps = psum2_pool.tile([128, H], dt.float32, tag="ps2")
            for hh in range(0, H, NSTEP):
                nc.tensor.matmul(b2_ps[:, hh:hh + NSTEP], ones1[:],
                                 b2_sb[:, hh:hh + NSTEP], start=True, stop=True)
            b2_bcast = const_pool.tile([128, H], dt.float32)
            nc.vector.tensor_copy(b2_bcast[:], b2_ps[:])

            # b1 per-partition [128, FC]
            b1_pp = const_pool.tile([128, FC], dt.float32)
            nc.sync.dma_start(
                b1_pp[:], b1_d[:].rearrange("o (c p) -> (o p) c", p=128))

            zero_row = const_pool.tile([128, H], dt.bfloat16)
            nc.vector.memset(zero_row[:], 0.0)

            # persistent routing outputs (filled by the gate phase)
            ids_pp = const_pool.tile([128, NCOLS], dt.int32)
            g_pp = const_pool.tile([128, NCOLS], dt.float32)

            # ================= gate phase (scoped pool) ================
            with (
                tc.tile_pool(name="gate", bufs=1) as gate_pool,
                tc.tile_pool(name="small", bufs=2) as small_pool,
            ):
                # PE warm-up: a few dependency-free matmuls ramp the tensor
                # engine out of its low p-state during the entry barrier, so
                # the latency-critical gate matmul runs at speed.
                warm_src = gate_pool.tile([128, 512], dt.float32)
                nc.vector.memset(warm_src[:], 0.0)
                for wi in range(8):
                    warm_ps = psum1_pool.tile([128, 512], dt.float32,
                                              tag="ps1")
                    nc.tensor.matmul(warm_ps[:], ones128[:], warm_src[:],
                                     start=True, stop=True)

                # gate inputs first on the sync queue: they head the z-score
                # critical path
                xT_sb = gate_pool.tile([128, HC, P], dt.float32)
                for ci in range(HC):
                    nc.sync.dma_start(
                        xT_sb[:, ci, :], xT_s[ci * 128:(ci + 1) * 128, :])
                wg_sb = gate_pool.tile([128, HC, E], dt.float32)
                nc.sync.dma_start(
                    wg_sb[:], Wg_d[:].rearrange("(c p) e -> p c e", p=128))

                # ---- expert weights + dense zero-fill stream early on the
                # Activation engine's DMA queue, overlapping the whole gate
                # phase without delaying the sync-queue gate DMAs ----
                w1_sb = w_pool.tile([128, HC, FF], dt.bfloat16)
                for ci in range(HC):
                    nc.scalar.dma_start(
                        w1_sb[:, ci, :], W1_d[ci * 128:(ci + 1) * 128, :])
                w2_sb = w_pool.tile([128, FC, H], dt.bfloat16)
                for fc in range(FC):
                    nc.scalar.dma_start(
                        w2_sb[:, fc, :], W2_d[fc * 128:(fc + 1) * 128, :])
                for i in range(N // 128):
                    nc.scalar.dma_start(dense_d[i * 128:(i + 1) * 128, :],
                                        zero_row[:])

                z_sb_loc = gate_pool.tile([E, P], dt.float32)
                for t0 in range(0, P, 512):
                    zw = min(512, P - t0)
                    z_ps = psum1_pool.tile([E, 512], dt.float32, tag="ps1")
                    for ci in range(HC):
                        nc.tensor.matmul(z_ps[:, :zw], wg_sb[:, ci, :],
                                         xT_sb[:, ci, t0:t0 + zw],
                                         start=(ci == 0), stop=(ci == HC - 1))
                    nc.vector.tensor_copy(z_sb_loc[:, t0:t0 + zw],
                                          z_ps[:, :zw])
                nc.sync.dma_start(z_loc_d[:], z_sb_loc[:])

                # core c receives every shard's scores for expert c
                nc.gpsimd.collective_compute(
                    "AllToAll", Alu.bypass, replica_groups=groups,
                    ins=[z_loc_d[:]], outs=[z_e_d[:]],
                )

                z_sb = gate_pool.tile([128, ZF], dt.float32)
                nc.sync.dma_start(
                    z_sb[:], z_e_d[:].rearrange("q t -> (q t)").rearrange(
                        "(p f) -> p f", p=128))
                # contiguous per-partition layout: (p, f) = token p*F16+f.
                # sparse_gather's scan order (f*16+p) then yields an
                # interleaved — but consistent — permutation of the selected
                # set, which is all the FFN needs.
                z16 = gate_pool.tile([16, F16], dt.float32)
                nc.sync.dma_start(
                    z16[:], z_e_d[:].rearrange("q t -> (q t)").rearrange(
                        "(p f) -> p f", p=16))

                # ---- fp32 multi-probe bisection for the k-th largest ----
                probe_iota = gate_pool.tile([128, NPROBE], dt.int32)
                nc.gpsimd.iota(probe_iota[:], pattern=[[1, NPROBE]], base=1,
                               channel_multiplier=0)
                probe_f = gate_pool.tile([128, NPROBE], dt.float32)
                nc.vector.tensor_copy(probe_f[:], probe_iota[:])

                lo = gate_pool.tile([128, 1], dt.float32)
                hi = gate_pool.tile([128, 1], dt.float32)
                nc.vector.memset(lo[:], -BISECT_BOUND)
                nc.vector.memset(hi[:], BISECT_BOUND)
                kf = float(K)
                inv = 1.0 / (NPROBE + 1)
                for _ in range(BISECT_ROUNDS):
                    dl = small_pool.tile([128, 1], dt.float32, tag="dl")
                    nc.vector.tensor_tensor(dl[:], hi[:], lo[:],
                                            op=Alu.subtract)
                    nc.vector.tensor_scalar(dl[:], dl[:], inv, None,
                                            op0=Alu.mult)
                    mids = small_pool.tile([128, NPROBE], dt.float32,
                                           tag="mids")
                    nc.vector.tensor_scalar(mids[:], probe_f[:], dl[:, :1],
                                            lo[:, :1], op0=Alu.mult,
                                            op1=Alu.add)
                    part = small_pool.tile([128, NPROBE], dt.float32,
                                           tag="part")
                    for i in range(NPROBE):
                        cmpf = small_pool.tile([128, ZF], dt.float32,
                                               tag="cmpf")
                        nc.vector.tensor_scalar(cmpf[:], z_sb[:],
                                                mids[:, i:i + 1], None,
                                                op0=Alu.is_ge)
                        nc.vector.tensor_reduce(part[:, i:i + 1], cmpf[:],
                                                axis=mybir.AxisListType.X,
                                                op=Alu.add)
                    cnt_ps = psum1_pool.tile([128, NPROBE], dt.float32,
                                             tag="ps1")
                    nc.tensor.matmul(cnt_ps[:], ones128[:], part[:],
                                     start=True, stop=True)
                    cnt = small_pool.tile([128, NPROBE], dt.float32,
                                          tag="cnt")
                    nc.vector.tensor_copy(cnt[:], cnt_ps[:])
                    gemask = small_pool.tile([128, NPROBE], dt.uint8,
                                             tag="gemask")
                    ltmask = small_pool.tile([128, NPROBE], dt.uint8,
                                             tag="ltmask")
                    nc.vector.tensor_scalar(gemask[:], cnt[:], kf, None,
                                            op0=Alu.is_ge)
                    nc.vector.tensor_scalar(ltmask[:], cnt[:], kf, None,
                                            op0=Alu.is_lt)
                    # lo <- max(lo, max{mids[i] : count[i] >= K})
                    mlo = small_pool.tile([128, NPROBE], dt.float32,
                                          tag="mlo")
                    nc.vector.memset(mlo[:], -3e38)
                    nc.vector.copy_predicated(mlo[:], gemask[:], mids[:])
                    lomax = small_pool.tile([128, 1], dt.float32, tag="lomax")
                    nc.vector.tensor_reduce(lomax[:], mlo[:],
                                            axis=mybir.AxisListType.X,
                                            op=Alu.max)
                    nc.vector.tensor_tensor(lo[:], lo[:], lomax[:],
                                            op=Alu.max)
                    # hi <- min(hi, min{mids[i] : count[i] < K})
                    mhi = small_pool.tile([128, NPROBE], dt.float32,
                                          tag="mhi")
                    nc.vector.memset(mhi[:], 3e38)
                    nc.vector.copy_predicated(mhi[:], ltmask[:], mids[:])
                    himin = small_pool.tile([128, 1], dt.float32, tag="himin")
                    nc.vector.tensor_reduce(himin[:], mhi[:],
                                            axis=mybir.AxisListType.X,
                                            op=Alu.min)
                    nc.vector.tensor_tensor(hi[:], hi[:], himin[:],
                                            op=Alu.min)

                # ---- selection mask + sparse_gather compaction ----
                ids16 = gate_pool.tile([16, F16], dt.int32)
                nc.gpsimd.iota(ids16[:], pattern=[[1, F16]], base=0,
                               channel_multiplier=F16)
                idsf16 = gate_pool.tile([16, F16], dt.float32)
                nc.vector.tensor_copy(idsf16[:], ids16[:])

                mask16 = gate_pool.tile([16, F16], dt.uint8)
                nc.vector.tensor_scalar(mask16[:], z16[:], lo[0:16, 0:1],
                                        None, op0=Alu.is_ge)
                g16 = gate_pool.tile([16, F16], dt.float32)
                nc.scalar.activation(g16[:], z16[:], Act.Sigmoid)

                idm = gate_pool.tile([16, F16], dt.float32)
                nc.vector.memset(idm[:], -1.0)
                nc.vector.copy_predicated(idm[:], mask16[:], idsf16[:])
                gm = gate_pool.tile([16, F16], dt.float32)
                nc.vector.memset(gm[:], -1.0)
                nc.vector.copy_predicated(gm[:], mask16[:], g16[:])

                idc = gate_pool.tile([16, KOUT], dt.float32)
                nf1 = gate_pool.tile([1, 1], dt.uint32)
                nc.gpsimd.sparse_gather(idc[:], idm[:], num_found=nf1[:])
                gc = gate_pool.tile([16, KOUT], dt.float32)
                nf2 = gate_pool.tile([1, 1], dt.uint32)
                nc.gpsimd.sparse_gather(gc[:], gm[:], num_found=nf2[:])

                # compacted slot q lives at (q%16, q//16) in [16, K/16];
                # transpose -> [K/16=128, 16]: column s holds slots
                # q in {s, s+16, ...}: a valid subtile permutation.
                assert K // 16 == 128 and NCOLS == 16
                idT_ps = ptrans_pool.tile([128, 16], dt.float32, tag="pt")
                nc.tensor.transpose(idT_ps[:], idc[:, 0:K // 16], ident16[:])
                nc.vector.tensor_copy(ids_pp[:], idT_ps[:])
                gT_ps = ptrans_pool.tile([128, 16], dt.float32, tag="pt")
                nc.tensor.transpose(gT_ps[:], gc[:, 0:K // 16], ident16[:])
                nc.vector.tensor_copy(g_pp[:], gT_ps[:])

            # ================= FFN phase ================
            with (
                tc.tile_pool(name="ext", bufs=2) as ext_pool,
                tc.tile_pool(name="ex", bufs=1) as ex_pool,
                tc.tile_pool(name="hid", bufs=1) as hid_pool,
                tc.tile_pool(name="out", bufs=2) as out_pool,
            ):
                for g in range(NG):
                    # gather selected token rows (token-major)
                    ex_tok = ext_pool.tile([128, SUBS, H], dt.bfloat16,
                                           tag="ext")
                    for s in range(SUBS):
                        nc.gpsimd.indirect_dma_start(
                            out=ex_tok[:, s, :],
                            out_offset=None,
                            in_=x_bf[:],
                            in_offset=bass.IndirectOffsetOnAxis(
                                ap=ids_pp[:, g * SUBS + s:g * SUBS + s + 1],
                                axis=0),
                        )

                    # transpose to [h, tok] layout for the PE
                    ex_T = ex_pool.tile([128, HC, TOKG], dt.bfloat16,
                                        tag="ex")
                    for s in range(SUBS):
                        for ci in range(HC):
                            pt = ptrans_pool.tile([128, 128], dt.bfloat16,
                                                  tag="pt")
                            nc.tensor.transpose(
                                pt[:], ex_tok[:, s, ci * 128:(ci + 1) * 128],
                                ident_bf[:])
                            nc.vector.tensor_copy(
                                ex_T[:, ci, s * 128:(s + 1) * 128], pt[:])

                    hid_sb = hid_pool.tile([128, FC, TOKG], dt.bfloat16,
                                           tag="hid")
                    for fc in range(FC):
                        ps1 = psum1_pool.tile([128, TOKG], dt.float32,
                                              tag="ps1")
                        for ci in range(HC):
                            nc.tensor.matmul(
                                ps1[:], w1_sb[:, ci, fc * 128:(fc + 1) * 128],
                                ex_T[:, ci, :],
                                start=(ci == 0), stop=(ci == HC - 1))
                        nc.scalar.activation(hid_sb[:, fc, :], ps1[:], act,
                                             bias=b1_pp[:, fc:fc + 1])

                    for s in range(SUBS):
                        col = g * SUBS + s
                        pso = psum2_pool.tile([128, H], dt.float32, tag="ps2")
                        for hh in range(0, H, NSTEP):
                            for fc in range(FC):
                                nc.tensor.matmul(
                                    pso[:, hh:hh + NSTEP],
                                    hid_sb[:, fc, s * 128:(s + 1) * 128],
                                    w2_sb[:, fc, hh:hh + NSTEP],
                                    start=(fc == 0), stop=(fc == FC - 1))
                        out_bf = out_pool.tile([128, H], dt.bfloat16,
                                               tag="obf")
                        nc.vector.tensor_tensor(out_bf[:], pso[:],
                                                b2_bcast[:], op=Alu.add)
                        nc.vector.tensor_scalar(out_bf[:], out_bf[:],
                                                g_pp[:, col:col + 1], None,
                                                op0=Alu.mult)
                        nc.gpsimd.indirect_dma_start(
                            out=dense_d[:],
                            out_offset=bass.IndirectOffsetOnAxis(
                                ap=ids_pp[:, col:col + 1], axis=0),
                            in_=out_bf[:],
                            in_offset=None,
                        )

                # ---------------- combine ----------------
                nc.gpsimd.collective_compute(
                    "ReduceScatter", Alu.add, replica_groups=groups,
                    ins=[dense_d[:]], outs=[rs_out_d[:]],
                )
                # final output copy split across both DMA queues
                nc.sync.dma_start(y_d[0:P // 2, :], rs_out_d[0:P // 2, :])
                nc.scalar.dma_start(y_d[P // 2:P, :], rs_out_d[P // 2:P, :])

    _insert_library_loads(nc)
    _split_excess_waits(nc)
    return nc


# ---------------------------------------------------------------------------
# host-side sharding + execution
# ---------------------------------------------------------------------------

def make_in_maps(x, Wg, W1, b1, W2, b2, N=8192, H=1024):
    xt = np.ascontiguousarray(x.reshape(N, H).astype(np.float32))
    x_bf = xt.astype(ml_dtypes.bfloat16)
    P = N // N_CORES
    in_maps = []
    for c in range(N_CORES):
        shard = xt[c * P:(c + 1) * P, :]
        in_maps.append({
            "xT_s": np.ascontiguousarray(shard.T),
            "x_bf": x_bf,
            "Wg": np.ascontiguousarray(Wg.astype(np.float32)),
            "W1": np.ascontiguousarray(W1[c].astype(ml_dtypes.bfloat16)),
            "W2": np.ascontiguousarray(W2[c].astype(ml_dtypes.bfloat16)),
            "b1": np.ascontiguousarray(b1[c].reshape(1, -1).astype(np.float32)),
            "b2": np.ascontiguousarray(b2[c].reshape(1, -1).astype(np.float32)),
        })
    return in_maps


_NC_CACHE = {}


def kernel(x, Wg, W1, b1, W2, b2):
    x = np.asarray(x)
    B, L, H = x.shape
    N = B * L
    FF = W1.shape[2]
    key = (N, H, FF)
    if key not in _NC_CACHE:
        _NC_CACHE[key] = build_moe_nc(N=N, H=H, FF=FF)
    nc = _NC_CACHE[key]
    in_maps = make_in_maps(np.asarray(x), np.asarray(Wg), np.asarray(W1),
                           np.asarray(b1), np.asarray(W2), np.asarray(b2),
                           N=N, H=H)
    from concourse.bass_utils import run_bass_kernel_spmd
    res = run_bass_kernel_spmd(nc, in_maps, core_ids=list(range(N_CORES)),
                               trace=False)
    out = np.concatenate(
        [np.asarray(res.results[c]["y"]).astype(np.float32)
         for c in range(N_CORES)], axis=0)
    return out.reshape(B, L, H)
